# revision 1
# baseline (speedup 1.0000x reference)
"""AdapterFormer block (MHA + 5-branch soft-MoE FFN) on 8 TRN2 NeuronCores.

Data-parallel over batch (dim 1 of x): 64 -> 8 per core, weights
replicated, zero collectives.  Per-core tokens: 1576 = 197*8 (l-major,
tok = l*8 + b), 13 partition-tiles of 128 (last tile 40 valid rows).

Activations alternate between token-major (tok on partitions: LN stats,
softmax routing, per-token combines, residuals) and feature-major (feat
on partitions: matmul contraction layout).  Matmul operands are bf16
(f32 PSUM accumulation); residuals and stats stay f32.  Big weights are
cast f32->bf16 into scratch DRAM once (SWDGE cast-DMA), then streamed
into SBUF transposed via XBAR transpose-DMA chunks.
"""
import os
import sys

import numpy as np

sys.path.insert(0, '/opt/trn_rl_repo')

import concourse.bass as bass
import concourse.mybir as mybir
import concourse.tile as tile
from concourse.bass_utils import run_bass_kernel_spmd
from concourse.masks import make_identity
from concourse.tile_rust import add_dep_helper

# Enable the LDWEIGHTS optimizer (hardcoded off in bass_utils): overlaps /
# elides stationary-operand loads, which otherwise cost ~100 ns per matmul.
if os.environ.get('BASS_LDW_OPT', '0') == '1':
    import concourse.bass_utils as _bu
    if not getattr(_bu, '_ldw_patched', False):
        _orig_run_command = _bu.run_command

        def _patched_run_command(cmd, *a, **k):
            cmd = ['--enable-ldw-opt=true' if c == '--enable-ldw-opt=false'
                   else c for c in cmd]
            return _orig_run_command(cmd, *a, **k)

        _bu.run_command = _patched_run_command
        _bu._ldw_patched = True

F32 = mybir.dt.float32
BF16 = mybir.dt.bfloat16
AX = mybir.AxisListType.X
OP = mybir.AluOpType
ACTF = mybir.ActivationFunctionType

L, BT, D = 197, 64, 768
H, HD = 12, 64
E = 4
E1 = E + 1
DF = 4 * D                       # 3072
NCORES = 8
B = BT // NCORES                 # 8
NT = L * B                       # 1576
TT = (NT + 127) // 128           # 13
DSUB = D // 128                  # 6
FSUB = DF // 128                 # 24
QKV3 = 3 * D
LROWS = NT - (TT - 1) * 128      # 40
EPS = 1e-5
SCALE = 0.125                    # 1/sqrt(HD)
NCH = [(0, 512), (512, 512), (1024, 512), (1536, NT - 1536)]  # tok chunks

_DEBUG = bool(int(os.environ.get('BASS_KERNEL_DEBUG', '0')))


def _rows(t):
    return 128 if t < TT - 1 else LROWS


# --------------------------------------------------------------------------
# walrus here rejects instructions with >1 sync-wait entry ("Too many sync
# wait commands"); move extra waits onto single-wait NoOps on the same queue.
_ws_ctr = [0]


def _fix_multiwait(nc, max_waits=1):
    for fn in nc.m.functions:
        for blk in fn.blocks:
            insts = list(blk.instructions)
            out = []
            changed = False
            for inst in insts:
                si = inst.sync_info
                waits = list(si.on_wait) if (si is not None and si.on_wait) else []
                if len(waits) > max_waits:
                    extra, keep = waits[:-max_waits], waits[-max_waits:]
                    for i in range(0, len(extra), max_waits):
                        _ws_ctr[0] += 1
                        out.append(mybir.InstNoOp(
                            name=f"I-ws{_ws_ctr[0]}",
                            sync_info=mybir.SyncInfo(
                                on_wait=list(extra[i:i + max_waits]),
                                on_update=[]),
                            bass_nofuse=True,
                            engine=inst.engine,
                        ))
                    si.on_wait = keep
                    inst.sync_info = si
                    changed = True
                out.append(inst)
            if changed:
                blk.instructions = out
    return nc


# --------------------------------------------------------------------------
def build():
    nc = bass.Bass("TRN2", target_bir_lowering=False, debug=False,
                   num_devices=NCORES)

    x_ext = nc.declare_dram_parameter("x", [L, B, D], F32, isOutput=False)
    w = {}
    for name, shape in [
        ("ln1_g", [D]), ("ln1_b", [D]),
        ("in_proj_w", [QKV3, D]), ("in_proj_b", [QKV3]),
        ("out_proj_w", [D, D]), ("out_proj_b", [D]),
        ("ln2_g", [D]), ("ln2_b", [D]),
        ("c_fc_w", [DF, D]), ("c_fc_b", [DF]),
        ("c_proj_w", [D, DF]), ("c_proj_b", [D]),
        ("eh_w", [E, DF, D]), ("eh_b", [E, DF]),
        ("et_w", [E, D, DF]), ("et_b", [E, D]),
        ("r1_w", [E1, D]), ("r1_b", [E1]),
        ("r2_w", [E1, DF]), ("r2_b", [E1]),
    ]:
        w[name] = nc.declare_dram_parameter(name, shape, F32, isOutput=False)
    out_ext = nc.declare_dram_parameter("out", [L, B, D], F32, isOutput=True)
    out_flat = out_ext[:].rearrange("l b d -> (l b) d")
    x_flat = x_ext[:].rearrange("l b d -> (l b) d")

    dbg = {}
    if _DEBUG:
        for name, shape in [
            ("dbg_ln1fm", [128, DSUB, NT]), ("dbg_qkv5", [128, 3, NT]),
            ("dbg_ofm", [128, DSUB, NT]), ("dbg_x1", [128, TT, D]),
            ("dbg_r1", [128, TT, E1]), ("dbg_oht", [128, FSUB, NT]),
            ("dbg_r2", [128, TT, E1]),
        ]:
            dbg[name] = nc.declare_dram_parameter(name, shape, F32,
                                                  isOutput=True)

    with tile.TileContext(nc) as tc:
        _body(nc, tc, x_flat, w, out_flat, dbg)

    _fix_multiwait(nc)
    return nc


def _body(nc, tc, x_flat, w, out_flat, dbg):
    from contextlib import ExitStack
    with ExitStack() as ctx:
        dram = ctx.enter_context(tc.tile_pool(name="dram", bufs=1,
                                              space="DRAM"))
        dramS = ctx.enter_context(tc.tile_pool(name="dramS", bufs=2,
                                               space="DRAM"))
        big = ctx.enter_context(tc.tile_pool(name="big", bufs=1))
        strm = ctx.enter_context(tc.tile_pool(name="strm", bufs=2))
        const = ctx.enter_context(tc.tile_pool(name="const", bufs=1))
        small = ctx.enter_context(tc.tile_pool(name="small", bufs=2))
        attnp = ctx.enter_context(tc.tile_pool(name="attnp", bufs=2))
        psA = ctx.enter_context(tc.tile_pool(name="psA", bufs=2,
                                             space="PSUM"))
        psB = ctx.enter_context(tc.tile_pool(name="psB", bufs=2,
                                             space="PSUM"))
        psT = ctx.enter_context(tc.tile_pool(name="psT", bufs=2,
                                             space="PSUM"))
        psS = ctx.enter_context(tc.tile_pool(name="psS", bufs=2,
                                             space="PSUM"))

        x1_dram = dram.tile([128, TT, D], F32)

        # ---- constants ----------------------------------------------------
        id_bf = const.tile([128, 128], BF16)
        make_identity(nc, id_bf[:])
        id_f32 = const.tile([128, 128], F32)
        make_identity(nc, id_f32[:])
        ones_bf = const.tile([1, 128], BF16)
        nc.vector.memset(ones_bf[:], 1.0)
        eps_col = const.tile([128, 1], F32)
        nc.vector.memset(eps_col[:], EPS)
        c1702 = const.tile([128, 1], F32)
        nc.vector.memset(c1702[:], 1.702)

        # ---- load x token-major (FIRST: its DMA descriptors must hit the
        # HWDGE rings before the ~52MB of weight-cast traffic, else LN1+QKV
        # stall ~230us behind them) -------------------------------------
        x_tm = big.tile([128, TT, D], F32, tag="xo")
        nc.vector.memset(x_tm[:, TT - 1, :], 0.0)
        x_load_insts = []
        for t in range(TT):
            r = _rows(t)
            x_load_insts.append(
                nc.scalar.dma_start(x_tm[0:r, t, :],
                                    x_flat[t * 128: t * 128 + r, :]))

        def load_cols_into(name, n, n0, dst, d0):
            # contiguous row load (scalar ring) + PE transposes: a scattered
            # per-partition DMA costs ~25 us of 4-byte descriptors on the
            # SWDGE queue and stalls the weight casts behind it.
            row = const.tile([1, 6 * 128], F32, tag="row_stage")
            nc.scalar.dma_start(row[0:1, 0:n * 128], w[name][:].rearrange(
                "(a d) -> a d", a=1)[0:1, n0 * 128:(n0 + n) * 128])
            for s in range(n):
                pt = psT.tile([128, 1], F32, tag="tp")
                nc.tensor.transpose(pt[:], row[0:1, s * 128:(s + 1) * 128],
                                    id_f32[0:1, 0:1])
                nc.vector.tensor_copy(dst[:, d0 + s:d0 + s + 1], pt[:])

        def load_cols(name, n):     # [dim] -> per-partition cols [128, n]
            t = const.tile([128, n], F32, tag=f"col_{name}")
            load_cols_into(name, n, 0, t, 0)
            return t

        ln1g = load_cols("ln1_g", DSUB)
        ln1b = load_cols("ln1_b", DSUB)
        ln2g = load_cols("ln2_g", DSUB)
        ln2b = load_cols("ln2_b", DSUB)
        # in_proj bias cols: scattered SWDGE DMA. A row+PE-transpose path
        # here convoys: it serializes through the single row_stage slot and
        # the in-order PE stream blocks on it deep into the attention phase.
        bqkv = const.tile([128, 18], F32)
        with nc.allow_non_contiguous_dma(reason="tiny per-partition col"):
            nc.gpsimd.dma_start(
                bqkv[:], w["in_proj_b"][:].rearrange("(s p) -> p s", p=128))
        # ---- layernorm helper --------------------------------------------
        lnt = big.tile([128, D], BF16, tag="lnt")

        def layernorm_to_fm(dst_fm, g_cols, b_cols):
            for t in range(TT):
                r = _rows(t)
                xs = x_tm[0:r, t, :]
                s1 = small.tile([128, 1], F32, tag="ln_s1")
                nc.vector.reduce_sum(s1[0:r, :], xs, AX)
                sq = small.tile([128, 1], F32, tag="ln_sq")
                xsq = small.tile([128, D], BF16, tag="ln_xsq")
                nc.scalar.activation(xsq[0:r, :], xs, ACTF.Square,
                                     accum_out=sq[0:r, :])
                mu = small.tile([128, 1], F32, tag="ln_mu")
                nc.vector.tensor_scalar(mu[0:r, :], s1[0:r, :], 1.0 / D,
                                        None, OP.mult)
                mu2 = small.tile([128, 1], F32, tag="ln_mu2")
                nc.vector.tensor_tensor(mu2[0:r, :], mu[0:r, :], mu[0:r, :],
                                        OP.mult)
                var = small.tile([128, 1], F32, tag="ln_var")
                nc.vector.scalar_tensor_tensor(
                    out=var[0:r, :], in0=sq[0:r, :], scalar=1.0 / D,
                    in1=mu2[0:r, :], op0=OP.mult, op1=OP.subtract)
                sd = small.tile([128, 1], F32, tag="ln_sd")
                nc.scalar.activation(sd[0:r, :], var[0:r, :], ACTF.Sqrt,
                                     bias=eps_col[0:r, :])
                a_col = small.tile([128, 1], F32, tag="ln_a")
                nc.vector.reciprocal(a_col[0:r, :], sd[0:r, :])
                b_col = small.tile([128, 1], F32, tag="ln_b")
                nc.vector.scalar_tensor_tensor(
                    out=b_col[0:r, :], in0=mu[0:r, :], scalar=-1.0,
                    in1=a_col[0:r, :], op0=OP.mult, op1=OP.mult)
                nc.scalar.activation(lnt[0:r, :], xs, ACTF.Identity,
                                     bias=b_col[0:r, :], scale=a_col[0:r, :])
                for s in range(DSUB):
                    pt = psT.tile([128, 128], BF16, tag="tp")
                    nc.tensor.transpose(pt[:, 0:r],
                                        lnt[0:r, s * 128:(s + 1) * 128],
                                        id_bf[0:r, 0:r])
                    nc.vector.tensor_scalar(
                        dst_fm[:, s, t * 128:t * 128 + r], pt[:, 0:r],
                        g_cols[:, s:s + 1], b_cols[:, s:s + 1],
                        OP.mult, OP.add)

        # ---- LN1 (emitted before the weight casts so it runs during them)
        ln_fm = big.tile([128, DSUB, NT], BF16, tag="ln")
        layernorm_to_fm(ln_fm, ln1g, ln1b)
        if _DEBUG:
            nc.gpsimd.dma_start(dbg["dbg_ln1fm"][:], ln_fm[:])

        opb_row = const.tile([1, D], BF16)
        nc.gpsimd.dma_start(opb_row[:], w["out_proj_b"][:].rearrange("(a d) -> a d", a=1))
        r1b_row = const.tile([1, E1], BF16)
        nc.gpsimd.dma_start(r1b_row[:], w["r1_b"][:].rearrange("(a e) -> a e", a=1))
        r2b_row = const.tile([1, E1], BF16)
        nc.gpsimd.dma_start(r2b_row[:], w["r2_b"][:].rearrange("(a e) -> a e", a=1))
        bt_stack = const.tile([E1, D], BF16)
        nc.gpsimd.dma_start(bt_stack[0:1, :],
                            w["c_proj_b"][:].rearrange("(a d) -> a d", a=1))
        nc.gpsimd.dma_start(bt_stack[1:, :], w["et_b"][:])

        # ---- bf16 weight scratch in DRAM (SWDGE cast-DMAs) ---------------
        # Only the QKV/out_proj weights are cast upfront (needed first).
        # The 44MB head/tail expert weights are cast per-chunk INSIDE the
        # head/tail loops: a bulk upfront cast head-of-line blocks the
        # shared HWDGE rings for ~250us and starves the wq transposes (the
        # first QKV matmul slipped to ~284us), while in-use-order chunk
        # casts are paced by the gpsimd engine stream + pool slots.
        wqkv_bf = dram.tile([QKV3, D], BF16)
        wout_bf = dram.tile([D, D], BF16)
        cast_insts = [
            nc.gpsimd.dma_start(wqkv_bf[:], w["in_proj_w"][:]),
            nc.gpsimd.dma_start(wout_bf[:], w["out_proj_w"][:]),
        ]
        for ci in cast_insts:
            add_dep_helper(ci.ins, x_load_insts[-1].ins, sync=True,
                           reason="weight casts wait for x load")

        ehw_flat = w["eh_w"][:].rearrange("e f d -> (e f) d")
        etw_flat = w["et_w"][:].rearrange("e d f -> (e d) f")

        r1_tm = const.tile([128, TT, E1], F32)
        r2_tm = const.tile([128, TT, E1], F32)
        r1T = const.tile([E1, NT], BF16)
        r2T = const.tile([E1, NT], BF16)

        # ---- QKV + attention, interleaved per head-pair tile mt ----------
        o_fm = big.tile([128, DSUB, NT], BF16, tag="oa")
        o_lb = o_fm[:].rearrange("p m (l b) -> p m l b", b=B)
        LT = [(0, 128), (128, L - 128)]
        for mt in range(DSUB):
            qkv5 = big.tile([128, 3, NT], BF16, tag="qk")
            for j, m in enumerate([mt, 6 + mt, 12 + mt]):
                wq = strm.tile([128, DSUB, 128], BF16, tag=f"wq{j}")
                nc.sync.dma_start_transpose(
                    wq[:], wqkv_bf[m * 128:(m + 1) * 128, :])
                for c0, cn in NCH:
                    pa = psA.tile([128, 512], F32, tag="mm")
                    for s in range(DSUB):
                        nc.tensor.matmul(pa[:, 0:cn], wq[:, s, :],
                                         ln_fm[:, s, c0:c0 + cn],
                                         start=(s == 0), stop=(s == DSUB - 1))
                    if j == 0:
                        nc.vector.tensor_scalar(
                            qkv5[:, j, c0:c0 + cn], pa[:, 0:cn],
                            bqkv[:, m:m + 1], SCALE, OP.add, OP.mult)
                    else:
                        nc.vector.tensor_scalar(
                            qkv5[:, j, c0:c0 + cn], pa[:, 0:cn],
                            bqkv[:, m:m + 1], None, OP.add)
            if _DEBUG and mt == 0:
                nc.gpsimd.dma_start(dbg["dbg_qkv5"][:], qkv5[:])
            qkv_lb = qkv5[:].rearrange("p j (l b) -> p j l b", b=B)
            v_all = small.tile([128, B, 2, 128], BF16, tag="v_tm")
            for b in range(B):
                vT2 = qkv_lb[:, 2, :, b]              # [128, 197] both heads
                for jj, (m0, mc) in enumerate(LT):
                    pt = psT.tile([128, 128], BF16, tag="tp")
                    nc.tensor.transpose(pt[0:mc, :], vT2[:, m0:m0 + mc],
                                        id_bf[:])
                    nc.vector.tensor_copy(v_all[0:mc, b, jj, :],
                                          pt[0:mc, :])
            for h in (2 * mt, 2 * mt + 1):
                po = (h % 2) * 64
                for b in range(B):
                    v_tm = v_all[:, b, :, :]
                    qT = qkv_lb[po:po + 64, 0, :, b]
                    kT = qkv_lb[po:po + 64, 1, :, b]
                    vT = qkv_lb[po:po + 64, 2, :, b]
                    attn = attnp.tile([128, 2, L], BF16, tag="attn")
                    rs = attnp.tile([128, 2], F32, tag="attn_rs")
                    for i, (l0, lc) in enumerate(LT):
                        ps = psS.tile([128, L], F32, tag="att")
                        nc.tensor.matmul(ps[0:lc, :], qT[:, l0:l0 + lc], kT,
                                         start=True, stop=True)
                        sums = attnp.tile([128, 1], F32, tag="attn_sum")
                        nc.scalar.activation(attn[0:lc, i, :], ps[0:lc, :],
                                             ACTF.Exp,
                                             accum_out=sums[0:lc, :])
                        nc.vector.reciprocal(rs[0:lc, i:i + 1],
                                             sums[0:lc, :])
                        nc.vector.tensor_scalar(
                            attn[0:lc, i, :], attn[0:lc, i, :],
                            rs[0:lc, i:i + 1], None, OP.mult)
                    attnT = attnp.tile([128, 2, L], BF16, tag="attnT")
                    for jj, (m0, mc) in enumerate(LT):
                        for i, (l0, lc) in enumerate(LT):
                            pt = psT.tile([128, 128], BF16, tag="tp")
                            nc.tensor.transpose(
                                pt[0:mc, 0:lc], attn[0:lc, i, m0:m0 + mc],
                                id_bf[0:lc, 0:lc])
                            nc.vector.tensor_copy(
                                attnT[0:mc, jj, l0:l0 + lc], pt[0:mc, 0:lc])
                    po_ps = psB.tile([64, L], F32, tag="mm2")
                    for jj, (m0, mc) in enumerate(LT):
                        nc.tensor.matmul(po_ps[:],
                                         v_tm[0:mc, jj, po:po + 64],
                                         attnT[0:mc, jj, :],
                                         start=(jj == 0), stop=(jj == 1))
                    nc.vector.tensor_copy(o_lb[po:po + 64, mt, :, b],
                                          po_ps[:])
        if _DEBUG:
            nc.gpsimd.dma_start(dbg["dbg_ofm"][:], o_fm[:])

        # ---- out_proj (token-major) + residual into x_tm ------------------
        # same 9KB slot shape as the tail s2a stream tiles; cycling through
        # that tag saves a resident 9KB/partition tag
        woutT = strm.tile([128, DSUB, D], BF16, tag="s2a")
        nc.sync.dma_start_transpose(woutT[:], wout_bf[:])
        DCH = [(0, 512), (512, 256)]
        for t in range(TT):
            r = _rows(t)
            for c0, cn in DCH:
                pa = psA.tile([128, 512], F32, tag="mm")
                for s in range(DSUB):
                    nc.tensor.matmul(
                        pa[0:r, 0:cn], o_fm[:, s, t * 128:t * 128 + r],
                        woutT[:, s, c0:c0 + cn],
                        start=(s == 0), stop=False)
                nc.tensor.matmul(pa[0:r, 0:cn], ones_bf[0:1, 0:r],
                                 opb_row[0:1, c0:c0 + cn],
                                 start=False, stop=True)
                nc.vector.tensor_tensor(
                    x_tm[0:r, t, c0:c0 + cn], pa[0:r, 0:cn],
                    x_tm[0:r, t, c0:c0 + cn], OP.add)
        if _DEBUG:
            nc.gpsimd.dma_start(dbg["dbg_x1"][:], x_tm[:])

        # ---- spill x1, LN2 ------------------------------------------------
        nc.scalar.dma_start(x1_dram[:], x_tm[:])
        ln2_fm = big.tile([128, DSUB, NT], BF16, tag="ln")
        layernorm_to_fm(ln2_fm, ln2g, ln2b)

        # routing weights transposed: [dsub*128, 5] via PE transpose
        # (streamed through a small 2-slot tag instead of resident copies)
        r1wT = const.tile([128, DSUB, E1], BF16)
        rw1 = strm.tile([E1, D], BF16, tag="rw")
        nc.gpsimd.dma_start(rw1[:], w["r1_w"][:])
        for s in range(DSUB):
            pt = psT.tile([128, E1], BF16, tag="tp")
            nc.tensor.transpose(pt[:], rw1[:, s * 128:(s + 1) * 128],
                                id_bf[0:E1, 0:E1])
            nc.vector.tensor_copy(r1wT[:, s, :], pt[:])
        r2wT = const.tile([128, FSUB, E1], BF16)
        for c in range(4):
            rw2 = strm.tile([E1, D], BF16, tag="rw")
            nc.gpsimd.dma_start(rw2[:], w["r2_w"][:][:, c * D:(c + 1) * D])
            for s6 in range(DSUB):
                pt = psT.tile([128, E1], BF16, tag="tp")
                nc.tensor.transpose(pt[:], rw2[:, s6 * 128:(s6 + 1) * 128],
                                    id_bf[0:E1, 0:E1])
                nc.vector.tensor_copy(r2wT[:, c * DSUB + s6, :], pt[:])

        # ---- routing helper (token-major logits, no max-sub: tiny logits)
        def routing(act_fm, nsub, wT, b_row, r_tm, rT):
            for t in range(TT):
                r = _rows(t)
                pr = psB.tile([128, 512], F32, tag="mm2")
                for s in range(nsub):
                    nc.tensor.matmul(pr[0:r, 0:E1],
                                     act_fm[:, s, t * 128:t * 128 + r],
                                     wT[:, s, :],
                                     start=(s == 0), stop=False)
                nc.tensor.matmul(pr[0:r, 0:E1], ones_bf[0:1, 0:r],
                                 b_row[0:1, :], start=False, stop=True)
                e_t = small.tile([128, E1], F32, tag="rt_exp")
                sums = small.tile([128, 1], F32, tag="rt_sum")
                nc.scalar.activation(e_t[0:r, :], pr[0:r, 0:E1], ACTF.Exp,
                                     accum_out=sums[0:r, :])
                rsum = small.tile([128, 1], F32, tag="rt_rsum")
                nc.vector.reciprocal(rsum[0:r, :], sums[0:r, :])
                nc.vector.tensor_scalar(r_tm[0:r, t, :], e_t[0:r, :],
                                        rsum[0:r, :], None, OP.mult)
                ptb = psT.tile([E1, 128], F32, tag="tp")
                nc.tensor.transpose(ptb[:, 0:r], r_tm[0:r, t, :],
                                    id_f32[0:r, 0:r])
                nc.vector.tensor_copy(rT[:, t * 128:t * 128 + r],
                                      ptb[:, 0:r])

        routing(ln2_fm, DSUB, r1wT, r1b_row, r1_tm, r1T)
        if _DEBUG:
            nc.gpsimd.dma_start(dbg["dbg_r1"][:], r1_tm[:])

        # ---- head stage ---------------------------------------------------
        # oh_s (token-major, bf16 accum) -> quickgelu -> transpose into oht
        oht = big.tile([128, FSUB, NT], BF16, tag="xo")
        oh_s = big.tile([128, TT, 512], BF16, tag="oa")
        for sl in range(DF // 512):
            # head biases streamed per 512-slice (replaces the resident
            # [E1, DF] bh_stack -- 6KB/partition of SBUF)
            bh = strm.tile([E1, 512], BF16, tag="bh")
            nc.gpsimd.dma_start(bh[0:1, :], w["c_fc_b"][:].rearrange(
                "(a f) -> a f", a=1)[0:1, sl * 512:(sl + 1) * 512])
            nc.gpsimd.dma_start(bh[1:, :],
                                w["eh_b"][:][:, sl * 512:(sl + 1) * 512])
            for t in range(TT):
                r = _rows(t)
                pb = psB.tile([128, 512], F32, tag="mm2")
                nc.tensor.matmul(pb[0:r, :], r1T[:, t * 128:t * 128 + r],
                                 bh[:, :], start=True, stop=True)
                nc.vector.tensor_copy(oh_s[0:r, t, :], pb[0:r, :])
            for e in range(E1):
                # just-in-time chunk cast from the original f32 weights
                whc = dramS.tile([512, D], BF16, tag="whc")
                if e == 0:
                    nc.gpsimd.dma_start(
                        whc[:], w["c_fc_w"][:][sl * 512:(sl + 1) * 512, :])
                else:
                    nc.gpsimd.dma_start(
                        whc[:],
                        ehw_flat[(e - 1) * DF + sl * 512:
                                 (e - 1) * DF + (sl + 1) * 512, :])
                wch = strm.tile([128, DSUB, 512], BF16, tag="s2a")
                nc.sync.dma_start_transpose(wch[:], whc[:])
                for t in range(TT):
                    r = _rows(t)
                    pa = psA.tile([128, 512], F32, tag="mm")
                    for s in range(DSUB):
                        nc.tensor.matmul(
                            pa[0:r, :],
                            ln2_fm[:, s, t * 128:t * 128 + r],
                            wch[:, s, :],
                            start=(s == 0), stop=(s == DSUB - 1))
                    nc.vector.scalar_tensor_tensor(
                        out=oh_s[0:r, t, :], in0=pa[0:r, :],
                        scalar=r1_tm[0:r, t, e:e + 1],
                        in1=oh_s[0:r, t, :], op0=OP.mult, op1=OP.add)
            for t in range(TT):
                r = _rows(t)
                sig = small.tile([128, 512], BF16, tag="sig")
                nc.scalar.activation(sig[0:r, :], oh_s[0:r, t, :],
                                     ACTF.Sigmoid, scale=c1702[0:r, :])
                nc.vector.tensor_tensor(oh_s[0:r, t, :], oh_s[0:r, t, :],
                                        sig[0:r, :], OP.mult)
                for j in range(4):
                    pt = psT.tile([128, 128], BF16, tag="tp")
                    nc.tensor.transpose(pt[:, 0:r],
                                        oh_s[0:r, t, j * 128:(j + 1) * 128],
                                        id_bf[0:r, 0:r])
                    nc.vector.tensor_copy(
                        oht[:, sl * 4 + j, t * 128:t * 128 + r], pt[:, 0:r])
        if _DEBUG:
            nc.gpsimd.dma_start(dbg["dbg_oht"][:], oht[:])

        # ---- r2 routing ---------------------------------------------------
        routing(oht, FSUB, r2wT, r2b_row, r2_tm, r2T)
        if _DEBUG:
            nc.gpsimd.dma_start(dbg["dbg_r2"][:], r2_tm[:])

        # ---- tail stage + residual + store -------------------------------
        out_s = big.tile([128, TT, 384], F32, tag="ln")
        for dsl in range(2):
            d0 = dsl * 384
            for t in range(TT):
                r = _rows(t)
                pb = psB.tile([128, 512], F32, tag="mm2")
                nc.tensor.matmul(pb[0:r, 0:384],
                                 r2T[:, t * 128:t * 128 + r],
                                 bt_stack[:, d0:d0 + 384],
                                 start=True, stop=True)
                x1s = small.tile([128, 384], F32, tag="x1s")
                nc.scalar.dma_start(x1s[0:r, :], x1_dram[0:r, t, d0:d0 + 384])
                nc.vector.tensor_tensor(out_s[0:r, t, :], pb[0:r, 0:384],
                                        x1s[0:r, :], OP.add)
            for e in range(E1):
                wtc = dramS.tile([384, DF], BF16, tag="wtc")
                if e == 0:
                    nc.gpsimd.dma_start(
                        wtc[:], w["c_proj_w"][:][d0:d0 + 384, :])
                else:
                    nc.gpsimd.dma_start(
                        wtc[:], etw_flat[(e - 1) * D + d0:
                                         (e - 1) * D + d0 + 384, :])
                # distinct tags: 4 stream buffers in flight so expert e+1's
                # weight DMA overlaps expert e's matmuls (was a 5us PE gap
                # + HAM re-throttle per expert)
                wch0 = strm.tile([128, 12, 384], BF16, tag="s2a")
                nc.sync.dma_start_transpose(wch0[:], wtc[0:384, 0:12 * 128])
                wch1 = strm.tile([128, 12, 384], BF16, tag="s2b")
                nc.sync.dma_start_transpose(wch1[:], wtc[0:384, 12 * 128:])
                for t in range(TT):
                    r = _rows(t)
                    pa = psA.tile([128, 512], F32, tag="mm")
                    for s in range(FSUB):
                        wc = wch0 if s < 12 else wch1
                        nc.tensor.matmul(
                            pa[0:r, 0:384],
                            oht[:, s, t * 128:t * 128 + r],
                            wc[:, s % 12, :],
                            start=(s == 0), stop=(s == FSUB - 1))
                    nc.vector.scalar_tensor_tensor(
                        out=out_s[0:r, t, :], in0=pa[0:r, 0:384],
                        scalar=r2_tm[0:r, t, e:e + 1],
                        in1=out_s[0:r, t, :], op0=OP.mult, op1=OP.add)
            for t in range(TT):
                r = _rows(t)
                nc.scalar.dma_start(
                    out_flat[t * 128:t * 128 + r, d0:d0 + 384],
                    out_s[0:r, t, :])


# --------------------------------------------------------------------------
_cache = {}


def _get_nc():
    if 'nc' not in _cache:
        _cache['nc'] = build()
    return _cache['nc']


def _run(inputs, trace=False, trace_kwargs=None):
    nc = _get_nc()
    full = {k: np.ascontiguousarray(np.asarray(v), dtype=np.float32)
            for k, v in inputs.items()}
    in_maps = []
    for c in range(NCORES):
        m = {k: v for k, v in full.items() if k != 'x'}
        m['x'] = np.ascontiguousarray(full['x'][:, c * B:(c + 1) * B, :])
        in_maps.append(m)
    res = run_bass_kernel_spmd(nc, in_maps, core_ids=list(range(NCORES)),
                               trace=trace, **(trace_kwargs or {}))
    out = np.concatenate([res.results[c]['out'] for c in range(NCORES)],
                         axis=1)
    return out, res


def kernel(**inputs) -> np.ndarray:
    out, _ = _run(inputs, trace=False)
    return out



# revision 19
# speedup vs baseline: 1.0538x; 1.0538x over previous
"""AdapterFormer block (MHA + 5-branch soft-MoE FFN) on 8 TRN2 NeuronCores.

Data-parallel over batch (dim 1 of x): 64 -> 8 per core, weights
replicated, zero collectives.  Per-core tokens: 1576 = 197*8 (l-major,
tok = l*8 + b), 13 partition-tiles of 128 (last tile 40 valid rows).

Activations alternate between token-major (tok on partitions: LN stats,
softmax routing, per-token combines, residuals) and feature-major (feat
on partitions: matmul contraction layout).  Matmul operands are bf16
(f32 PSUM accumulation); residuals and stats stay f32.  Big weights are
cast f32->bf16 into scratch DRAM once (SWDGE cast-DMA), then streamed
into SBUF transposed via XBAR transpose-DMA chunks.
"""
import os
import sys

import numpy as np

sys.path.insert(0, '/opt/trn_rl_repo')

import concourse.bass as bass
import concourse.mybir as mybir
import concourse.tile as tile
from concourse.bass_utils import run_bass_kernel_spmd
from concourse.masks import make_identity
from concourse.tile_rust import add_dep_helper

# Enable the LDWEIGHTS optimizer (hardcoded off in bass_utils): overlaps /
# elides stationary-operand loads, which otherwise cost ~100 ns per matmul.
if os.environ.get('BASS_LDW_OPT', '0') == '1':
    import concourse.bass_utils as _bu
    if not getattr(_bu, '_ldw_patched', False):
        _orig_run_command = _bu.run_command

        def _patched_run_command(cmd, *a, **k):
            cmd = ['--enable-ldw-opt=true' if c == '--enable-ldw-opt=false'
                   else c for c in cmd]
            return _orig_run_command(cmd, *a, **k)

        _bu.run_command = _patched_run_command
        _bu._ldw_patched = True

F32 = mybir.dt.float32
BF16 = mybir.dt.bfloat16
F8E4 = mybir.dt.float8e4
AX = mybir.AxisListType.X
OP = mybir.AluOpType
ACTF = mybir.ActivationFunctionType

L, BT, D = 197, 64, 768
H, HD = 12, 64
E = 4
E1 = E + 1
DF = 4 * D                       # 3072
NCORES = 8
B = BT // NCORES                 # 8
NT = L * B                       # 1576
TT = (NT + 127) // 128           # 13
DSUB = D // 128                  # 6
FSUB = DF // 128                 # 24
QKV3 = 3 * D
LROWS = NT - (TT - 1) * 128      # 40
EPS = 1e-5
SCALE = 0.125                    # 1/sqrt(HD)
NCH = [(0, 512), (512, 512), (1024, 512), (1536, NT - 1536)]  # tok chunks

_DEBUG = bool(int(os.environ.get('BASS_KERNEL_DEBUG', '0')))


def _rows(t):
    return 128 if t < TT - 1 else LROWS


# --------------------------------------------------------------------------
# walrus here rejects instructions with >1 sync-wait entry ("Too many sync
# wait commands"); move extra waits onto single-wait NoOps on the same queue.
_ws_ctr = [0]


def _fix_multiwait(nc, max_waits=1):
    for fn in nc.m.functions:
        for blk in fn.blocks:
            insts = list(blk.instructions)
            out = []
            changed = False
            for inst in insts:
                si = inst.sync_info
                waits = list(si.on_wait) if (si is not None and si.on_wait) else []
                if len(waits) > max_waits:
                    extra, keep = waits[:-max_waits], waits[-max_waits:]
                    for i in range(0, len(extra), max_waits):
                        _ws_ctr[0] += 1
                        out.append(mybir.InstNoOp(
                            name=f"I-ws{_ws_ctr[0]}",
                            sync_info=mybir.SyncInfo(
                                on_wait=list(extra[i:i + max_waits]),
                                on_update=[]),
                            bass_nofuse=True,
                            engine=inst.engine,
                        ))
                    si.on_wait = keep
                    inst.sync_info = si
                    changed = True
                out.append(inst)
            if changed:
                blk.instructions = out
    return nc


# --------------------------------------------------------------------------
def build():
    nc = bass.Bass("TRN2", target_bir_lowering=False, debug=False,
                   num_devices=NCORES)

    x_ext = nc.declare_dram_parameter("x", [L, B, D], F32, isOutput=False)
    w = {}
    for name, shape in [
        ("ln1_g", [D]), ("ln1_b", [D]),
        ("in_proj_w", [QKV3, D]), ("in_proj_b", [QKV3]),
        ("out_proj_w", [D, D]), ("out_proj_b", [D]),
        ("ln2_g", [D]), ("ln2_b", [D]),
        ("c_fc_w", [DF, D]), ("c_fc_b", [DF]),
        ("c_proj_w", [D, DF]), ("c_proj_b", [D]),
        ("eh_w", [E, DF, D]), ("eh_b", [E, DF]),
        ("et_w", [E, D, DF]), ("et_b", [E, D]),
        ("r1_w", [E1, D]), ("r1_b", [E1]),
        ("r2_w", [E1, DF]), ("r2_b", [E1]),
    ]:
        w[name] = nc.declare_dram_parameter(name, shape, F32, isOutput=False)
    out_ext = nc.declare_dram_parameter("out", [L, B, D], F32, isOutput=True)
    out_flat = out_ext[:].rearrange("l b d -> (l b) d")
    x_flat = x_ext[:].rearrange("l b d -> (l b) d")

    dbg = {}
    if _DEBUG:
        for name, shape in [
            ("dbg_ln1fm", [128, DSUB, NT]), ("dbg_qkv5", [128, 3, NT]),
            ("dbg_ofm", [128, DSUB, NT]), ("dbg_x1", [128, TT, D]),
            ("dbg_r1", [128, TT, E1]), ("dbg_oht", [128, FSUB, NT]),
            ("dbg_r2", [128, TT, E1]),
        ]:
            dbg[name] = nc.declare_dram_parameter(name, shape, F32,
                                                  isOutput=True)

    with tile.TileContext(nc) as tc:
        _body(nc, tc, x_flat, w, out_flat, dbg)

    _fix_multiwait(nc)
    return nc


def _body(nc, tc, x_flat, w, out_flat, dbg):
    from contextlib import ExitStack
    with ExitStack() as ctx:
        dram = ctx.enter_context(tc.tile_pool(name="dram", bufs=1,
                                              space="DRAM"))
        dramS = ctx.enter_context(tc.tile_pool(name="dramS", bufs=2,
                                               space="DRAM"))
        big = ctx.enter_context(tc.tile_pool(name="big", bufs=1))
        strm = ctx.enter_context(tc.tile_pool(name="strm", bufs=2))
        const = ctx.enter_context(tc.tile_pool(name="const", bufs=1))
        small = ctx.enter_context(tc.tile_pool(name="small", bufs=2))
        attnp = ctx.enter_context(tc.tile_pool(name="attnp", bufs=2))
        psA = ctx.enter_context(tc.tile_pool(name="psA", bufs=2,
                                             space="PSUM"))
        psB = ctx.enter_context(tc.tile_pool(name="psB", bufs=2,
                                             space="PSUM"))
        psT = ctx.enter_context(tc.tile_pool(name="psT", bufs=2,
                                             space="PSUM"))
        psS = ctx.enter_context(tc.tile_pool(name="psS", bufs=2,
                                             space="PSUM"))
        wrows = ctx.enter_context(tc.tile_pool(name="wrows", bufs=3))
        crows = ctx.enter_context(tc.tile_pool(name="crows", bufs=2))

        x1_dram = dram.tile([128, TT, D], F32)

        # ---- constants ----------------------------------------------------
        id_bf = const.tile([128, 128], BF16)
        make_identity(nc, id_bf[:])
        id_f32 = const.tile([128, 128], F32)
        make_identity(nc, id_f32[:])
        ones_bf = const.tile([1, 128], BF16)
        nc.vector.memset(ones_bf[:], 1.0)
        eps_col = const.tile([128, 1], F32)
        nc.vector.memset(eps_col[:], EPS)
        c1702 = const.tile([128, 1], F32)
        nc.vector.memset(c1702[:], 1.702)

        # ---- tiny const rows first (scalar ring, ~15KB total: lands in
        # ~2us so their PE transposes never gate on the bulk x traffic).
        # Streamed through a 2-slot [1, D] pool; in_proj_b goes in 3 chunks.
        def row_chunk(name, c0):
            row = crows.tile([1, D], F32, tag="crow")
            nc.scalar.dma_start(row[:], w[name][:].rearrange(
                "(a d) -> a d", a=1)[0:1, c0 * D:(c0 + 1) * D])
            return row

        def cols_from(row, dst, s0, n):
            for s in range(n):
                pt = psT.tile([128, 1], F32, tag="tp")
                nc.tensor.transpose(pt[:], row[0:1, s * 128:(s + 1) * 128],
                                    id_f32[0:1, 0:1])
                nc.vector.tensor_copy(dst[:, s0 + s:s0 + s + 1], pt[:])

        row_ln1g = row_chunk("ln1_g", 0)
        row_ln1b = row_chunk("ln1_b", 0)
        rows_ipb = [row_chunk("in_proj_b", c) for c in range(3)]

        # ---- load x token-major, split across BOTH HWDGE rings (a single
        # ring moves 5.1MB at ~45GB/s = 113us and gated LN1 by that long) --
        x_tm = big.tile([128, TT, D], F32, tag="xo")
        nc.vector.memset(x_tm[:, TT - 1, :], 0.0)
        for t in range(TT):
            r = _rows(t)
            eng = nc.scalar if t % 2 == 0 else nc.sync
            eng.dma_start(x_tm[0:r, t, :], x_flat[t * 128: t * 128 + r, :])

        # ln2 g/b rows: behind the x halves on the scalar ring (~58us) --
        # their col transposes are emitted just before LN2 (~880us).
        row_ln2g = row_chunk("ln2_g", 0)
        row_ln2b = row_chunk("ln2_b", 0)

        # ---- QKV + out_proj weight rows as f32 on the gpsimd ring (idle at
        # start), streamed through a 4-slot pool in consumption order.
        # PE-transposed on-chip later: the old DRAM bf16 cast + XBAR
        # transpose-DMA path fed the first matmuls ~300us late (XBAR ring is
        # packet-rate-bound at ~30GB/s). -----------------------------------
        M_ORDER = [mt + 6 * j for mt in range(DSUB) for j in range(3)]
        wrow_t = {}
        for m in M_ORDER:
            wt = wrows.tile([128, D], F32, tag="wrow")
            nc.gpsimd.dma_start(wt[:],
                                w["in_proj_w"][:][m * 128:(m + 1) * 128, :])
            wrow_t[m] = wt
        wout_t = {}
        for m in range(DSUB):
            wt = wrows.tile([128, D], F32, tag="wrow")
            nc.gpsimd.dma_start(wt[:],
                                w["out_proj_w"][:][m * 128:(m + 1) * 128, :])
            wout_t[m] = wt

        ln1g = const.tile([128, DSUB], F32, tag="col_ln1g")
        cols_from(row_ln1g, ln1g, 0, DSUB)
        ln1b = const.tile([128, DSUB], F32, tag="col_ln1b")
        cols_from(row_ln1b, ln1b, 0, DSUB)
        bqkv = const.tile([128, 18], F32, tag="col_bqkv")
        for c in range(3):
            cols_from(rows_ipb[c], bqkv, 6 * c, DSUB)
        # ---- layernorm helper --------------------------------------------
        lnt = big.tile([128, D], BF16, tag="lnt")

        def layernorm_to_fm(dst_fm, g_cols, b_cols):
            for t in range(TT):
                r = _rows(t)
                xs = x_tm[0:r, t, :]
                s1 = small.tile([128, 1], F32, tag="ln_s1")
                nc.vector.reduce_sum(s1[0:r, :], xs, AX)
                sq = small.tile([128, 1], F32, tag="ln_sq")
                # Square's elementwise output is throwaway (only accum_out
                # matters); dump it into lnt, overwritten below anyway.
                nc.scalar.activation(lnt[0:r, :], xs, ACTF.Square,
                                     accum_out=sq[0:r, :])
                mu = small.tile([128, 1], F32, tag="ln_mu")
                nc.vector.tensor_scalar(mu[0:r, :], s1[0:r, :], 1.0 / D,
                                        None, OP.mult)
                mu2 = small.tile([128, 1], F32, tag="ln_mu2")
                nc.vector.tensor_tensor(mu2[0:r, :], mu[0:r, :], mu[0:r, :],
                                        OP.mult)
                var = small.tile([128, 1], F32, tag="ln_var")
                nc.vector.scalar_tensor_tensor(
                    out=var[0:r, :], in0=sq[0:r, :], scalar=1.0 / D,
                    in1=mu2[0:r, :], op0=OP.mult, op1=OP.subtract)
                sd = small.tile([128, 1], F32, tag="ln_sd")
                nc.scalar.activation(sd[0:r, :], var[0:r, :], ACTF.Sqrt,
                                     bias=eps_col[0:r, :])
                a_col = small.tile([128, 1], F32, tag="ln_a")
                nc.vector.reciprocal(a_col[0:r, :], sd[0:r, :])
                b_col = small.tile([128, 1], F32, tag="ln_b")
                nc.vector.scalar_tensor_tensor(
                    out=b_col[0:r, :], in0=mu[0:r, :], scalar=-1.0,
                    in1=a_col[0:r, :], op0=OP.mult, op1=OP.mult)
                nc.scalar.activation(lnt[0:r, :], xs, ACTF.Identity,
                                     bias=b_col[0:r, :], scale=a_col[0:r, :])
                for s in range(DSUB):
                    pt = psT.tile([128, 128], BF16, tag="tp")
                    nc.tensor.transpose(pt[:, 0:r],
                                        lnt[0:r, s * 128:(s + 1) * 128],
                                        id_bf[0:r, 0:r])
                    nc.vector.tensor_scalar(
                        dst_fm[:, s, t * 128:t * 128 + r], pt[:, 0:r],
                        g_cols[:, s:s + 1], b_cols[:, s:s + 1],
                        OP.mult, OP.add)

        # ---- LN1 (emitted before the weight casts so it runs during them)
        ln_fm = big.tile([128, DSUB, NT], BF16, tag="ln")
        layernorm_to_fm(ln_fm, ln1g, ln1b)
        if _DEBUG:
            nc.gpsimd.dma_start(dbg["dbg_ln1fm"][:], ln_fm[:])

        opb_row = const.tile([1, D], BF16)
        nc.gpsimd.dma_start(opb_row[:], w["out_proj_b"][:].rearrange("(a d) -> a d", a=1))
        r1b_row = const.tile([1, E1], BF16)
        nc.gpsimd.dma_start(r1b_row[:], w["r1_b"][:].rearrange("(a e) -> a e", a=1))
        r2b_row = const.tile([1, E1], BF16)
        nc.gpsimd.dma_start(r2b_row[:], w["r2_b"][:].rearrange("(a e) -> a e", a=1))
        bt_stack = const.tile([E1, D], BF16)
        nc.gpsimd.dma_start(bt_stack[0:1, :],
                            w["c_proj_b"][:].rearrange("(a d) -> a d", a=1))
        nc.gpsimd.dma_start(bt_stack[1:, :], w["et_b"][:])

        ehw_flat = w["eh_w"][:].rearrange("e f d -> (e f) d")
        etw_flat = w["et_w"][:].rearrange("e d f -> (e d) f")

        r1_tm = const.tile([128, TT, E1], F32)
        r2_tm = const.tile([128, TT, E1], F32)
        r1T = const.tile([E1, NT], BF16)
        r2T = const.tile([E1, NT], BF16)

        # ---- QKV + attention, interleaved per head-pair tile mt ----------
        o_fm = big.tile([128, DSUB, NT], BF16, tag="oa")
        o_lb = o_fm[:].rearrange("p m (l b) -> p m l b", b=B)
        LT = [(0, 128), (128, L - 128)]
        for mt in range(DSUB):
            qkv5 = big.tile([128, 3, NT], BF16, tag="qk")
            for j, m in enumerate([mt, 6 + mt, 12 + mt]):
                # PE-transpose the f32 weight rows into matmul layout
                # (f32 -> bf16 conversion rides the PSUM-drain copy)
                wq = strm.tile([128, DSUB, 128], BF16, tag=f"wq{j}")
                for s in range(DSUB):
                    pt = psT.tile([128, 128], F32, tag="tp")
                    nc.tensor.transpose(
                        pt[:], wrow_t[m][:, s * 128:(s + 1) * 128],
                        id_f32[:])
                    nc.vector.tensor_copy(wq[:, s, :], pt[:])
                for c0, cn in NCH:
                    pa = psA.tile([128, 512], F32, tag="mm")
                    for s in range(DSUB):
                        nc.tensor.matmul(pa[:, 0:cn], wq[:, s, :],
                                         ln_fm[:, s, c0:c0 + cn],
                                         start=(s == 0), stop=(s == DSUB - 1))
                    if j == 0:
                        nc.vector.tensor_scalar(
                            qkv5[:, j, c0:c0 + cn], pa[:, 0:cn],
                            bqkv[:, m:m + 1], SCALE, OP.add, OP.mult)
                    else:
                        nc.vector.tensor_scalar(
                            qkv5[:, j, c0:c0 + cn], pa[:, 0:cn],
                            bqkv[:, m:m + 1], None, OP.add)
            if _DEBUG and mt == 0:
                nc.gpsimd.dma_start(dbg["dbg_qkv5"][:], qkv5[:])
            qkv_lb = qkv5[:].rearrange("p j (l b) -> p j l b", b=B)
            v_all = small.tile([128, B, 2, 128], BF16, tag="v_tm")
            for b in range(B):
                vT2 = qkv_lb[:, 2, :, b]              # [128, 197] both heads
                for jj, (m0, mc) in enumerate(LT):
                    pt = psT.tile([128, 128], BF16, tag="tp")
                    nc.tensor.transpose(pt[0:mc, :], vT2[:, m0:m0 + mc],
                                        id_bf[:])
                    nc.vector.tensor_copy(v_all[0:mc, b, jj, :],
                                          pt[0:mc, :])
            for h in (2 * mt, 2 * mt + 1):
                po = (h % 2) * 64
                for b in range(B):
                    v_tm = v_all[:, b, :, :]
                    qT = qkv_lb[po:po + 64, 0, :, b]
                    kT = qkv_lb[po:po + 64, 1, :, b]
                    vT = qkv_lb[po:po + 64, 2, :, b]
                    attn = attnp.tile([128, 2, L], BF16, tag="attn")
                    rs = attnp.tile([128, 2], F32, tag="attn_rs")
                    for i, (l0, lc) in enumerate(LT):
                        ps = psS.tile([128, L], F32, tag="att")
                        nc.tensor.matmul(ps[0:lc, :], qT[:, l0:l0 + lc], kT,
                                         start=True, stop=True)
                        sums = attnp.tile([128, 1], F32, tag="attn_sum")
                        nc.scalar.activation(attn[0:lc, i, :], ps[0:lc, :],
                                             ACTF.Exp,
                                             accum_out=sums[0:lc, :])
                        nc.vector.reciprocal(rs[0:lc, i:i + 1],
                                             sums[0:lc, :])
                        nc.vector.tensor_scalar(
                            attn[0:lc, i, :], attn[0:lc, i, :],
                            rs[0:lc, i:i + 1], None, OP.mult)
                    attnT = attnp.tile([128, 2, L], BF16, tag="attnT")
                    for jj, (m0, mc) in enumerate(LT):
                        for i, (l0, lc) in enumerate(LT):
                            pt = psT.tile([128, 128], BF16, tag="tp")
                            nc.tensor.transpose(
                                pt[0:mc, 0:lc], attn[0:lc, i, m0:m0 + mc],
                                id_bf[0:lc, 0:lc])
                            nc.vector.tensor_copy(
                                attnT[0:mc, jj, l0:l0 + lc], pt[0:mc, 0:lc])
                    po_ps = psB.tile([64, L], F32, tag="mm2")
                    for jj, (m0, mc) in enumerate(LT):
                        nc.tensor.matmul(po_ps[:],
                                         v_tm[0:mc, jj, po:po + 64],
                                         attnT[0:mc, jj, :],
                                         start=(jj == 0), stop=(jj == 1))
                    nc.vector.tensor_copy(o_lb[po:po + 64, mt, :, b],
                                          po_ps[:])
        if _DEBUG:
            nc.gpsimd.dma_start(dbg["dbg_ofm"][:], o_fm[:])

        # ---- out_proj (token-major) + residual into x_tm ------------------
        woutT = strm.tile([128, DSUB, D], BF16, tag="s2a")
        for mrow in range(DSUB):
            for s in range(DSUB):
                pt = psT.tile([128, 128], F32, tag="tp")
                nc.tensor.transpose(
                    pt[:], wout_t[mrow][:, s * 128:(s + 1) * 128],
                    id_f32[:])
                nc.vector.tensor_copy(
                    woutT[:, s, mrow * 128:(mrow + 1) * 128], pt[:])
        DCH = [(0, 512), (512, 256)]
        for t in range(TT):
            r = _rows(t)
            for c0, cn in DCH:
                pa = psA.tile([128, 512], F32, tag="mm")
                for s in range(DSUB):
                    nc.tensor.matmul(
                        pa[0:r, 0:cn], o_fm[:, s, t * 128:t * 128 + r],
                        woutT[:, s, c0:c0 + cn],
                        start=(s == 0), stop=False)
                nc.tensor.matmul(pa[0:r, 0:cn], ones_bf[0:1, 0:r],
                                 opb_row[0:1, c0:c0 + cn],
                                 start=False, stop=True)
                nc.vector.tensor_tensor(
                    x_tm[0:r, t, c0:c0 + cn], pa[0:r, 0:cn],
                    x_tm[0:r, t, c0:c0 + cn], OP.add)
        if _DEBUG:
            nc.gpsimd.dma_start(dbg["dbg_x1"][:], x_tm[:])

        # ---- spill x1, LN2 ------------------------------------------------
        ln2g = const.tile([128, DSUB], F32, tag="col_ln2g")
        cols_from(row_ln2g, ln2g, 0, DSUB)
        ln2b = const.tile([128, DSUB], F32, tag="col_ln2b")
        cols_from(row_ln2b, ln2b, 0, DSUB)
        nc.scalar.dma_start(x1_dram[:], x_tm[:])
        ln2_fm = big.tile([128, DSUB, NT], BF16, tag="ln")
        layernorm_to_fm(ln2_fm, ln2g, ln2b)

        # routing weights transposed: [dsub*128, 5] via PE transpose
        # (streamed through a small 2-slot tag instead of resident copies)
        r1wT = const.tile([128, DSUB, E1], BF16)
        rw1 = strm.tile([E1, D], BF16, tag="rw")
        nc.gpsimd.dma_start(rw1[:], w["r1_w"][:])
        for s in range(DSUB):
            pt = psT.tile([128, E1], BF16, tag="tp")
            nc.tensor.transpose(pt[:], rw1[:, s * 128:(s + 1) * 128],
                                id_bf[0:E1, 0:E1])
            nc.vector.tensor_copy(r1wT[:, s, :], pt[:])
        r2wT = const.tile([128, FSUB, E1], BF16)
        for c in range(4):
            rw2 = strm.tile([E1, D], BF16, tag="rw")
            nc.gpsimd.dma_start(rw2[:], w["r2_w"][:][:, c * D:(c + 1) * D])
            for s6 in range(DSUB):
                pt = psT.tile([128, E1], BF16, tag="tp")
                nc.tensor.transpose(pt[:], rw2[:, s6 * 128:(s6 + 1) * 128],
                                    id_bf[0:E1, 0:E1])
                nc.vector.tensor_copy(r2wT[:, c * DSUB + s6, :], pt[:])

        # ---- routing helper (token-major logits, no max-sub: tiny logits)
        def routing(act_fm, nsub, wT, b_row, r_tm, rT):
            for t in range(TT):
                r = _rows(t)
                pr = psB.tile([128, 512], F32, tag="mm2")
                for s in range(nsub):
                    nc.tensor.matmul(pr[0:r, 0:E1],
                                     act_fm[:, s, t * 128:t * 128 + r],
                                     wT[:, s, :],
                                     start=(s == 0), stop=False)
                nc.tensor.matmul(pr[0:r, 0:E1], ones_bf[0:1, 0:r],
                                 b_row[0:1, :], start=False, stop=True)
                e_t = small.tile([128, E1], F32, tag="rt_exp")
                sums = small.tile([128, 1], F32, tag="rt_sum")
                nc.scalar.activation(e_t[0:r, :], pr[0:r, 0:E1], ACTF.Exp,
                                     accum_out=sums[0:r, :])
                rsum = small.tile([128, 1], F32, tag="rt_rsum")
                nc.vector.reciprocal(rsum[0:r, :], sums[0:r, :])
                nc.vector.tensor_scalar(r_tm[0:r, t, :], e_t[0:r, :],
                                        rsum[0:r, :], None, OP.mult)
                ptb = psT.tile([E1, 128], F32, tag="tp")
                nc.tensor.transpose(ptb[:, 0:r], r_tm[0:r, t, :],
                                    id_f32[0:r, 0:r])
                nc.vector.tensor_copy(rT[:, t * 128:t * 128 + r],
                                      ptb[:, 0:r])

        routing(ln2_fm, DSUB, r1wT, r1b_row, r1_tm, r1T)
        if _DEBUG:
            nc.gpsimd.dma_start(dbg["dbg_r1"][:], r1_tm[:])

        # ---- head stage ---------------------------------------------------
        # oh_s (token-major, bf16 accum) -> quickgelu -> transpose into oht
        # fp8 head activations: halves the "xo" SBUF slot (shared with the
        # f32 x_tm) and the tail matmul moving-operand footprint; gelu
        # outputs are ~N(0, 0.3) so e4m3 adds only ~3% elementwise noise on
        # the tail branch (~7e-3 on the final output, budget is 2e-2).
        oht = big.tile([128, FSUB, NT], F8E4, tag="xo")
        oh_s = big.tile([128, TT, 512], BF16, tag="oa")
        for sl in range(DF // 512):
            # head biases streamed per 512-slice (replaces the resident
            # [E1, DF] bh_stack -- 6KB/partition of SBUF)
            bh = strm.tile([E1, 512], BF16, tag="bh")
            nc.gpsimd.dma_start(bh[0:1, :], w["c_fc_b"][:].rearrange(
                "(a f) -> a f", a=1)[0:1, sl * 512:(sl + 1) * 512])
            nc.gpsimd.dma_start(bh[1:, :],
                                w["eh_b"][:][:, sl * 512:(sl + 1) * 512])
            for t in range(TT):
                r = _rows(t)
                pb = psB.tile([128, 512], F32, tag="mm2")
                nc.tensor.matmul(pb[0:r, :], r1T[:, t * 128:t * 128 + r],
                                 bh[:, :], start=True, stop=True)
                nc.vector.tensor_copy(oh_s[0:r, t, :], pb[0:r, :])
            for e in range(E1):
                # just-in-time chunk cast from the original f32 weights
                whc = dramS.tile([512, D], BF16, tag="whc")
                if e == 0:
                    nc.gpsimd.dma_start(
                        whc[:], w["c_fc_w"][:][sl * 512:(sl + 1) * 512, :])
                else:
                    nc.gpsimd.dma_start(
                        whc[:],
                        ehw_flat[(e - 1) * DF + sl * 512:
                                 (e - 1) * DF + (sl + 1) * 512, :])
                wch = strm.tile([128, DSUB, 512], BF16, tag="s2a")
                nc.sync.dma_start_transpose(wch[:], whc[:])
                for t in range(TT):
                    r = _rows(t)
                    pa = psA.tile([128, 512], F32, tag="mm")
                    for s in range(DSUB):
                        nc.tensor.matmul(
                            pa[0:r, :],
                            ln2_fm[:, s, t * 128:t * 128 + r],
                            wch[:, s, :],
                            start=(s == 0), stop=(s == DSUB - 1))
                    nc.vector.scalar_tensor_tensor(
                        out=oh_s[0:r, t, :], in0=pa[0:r, :],
                        scalar=r1_tm[0:r, t, e:e + 1],
                        in1=oh_s[0:r, t, :], op0=OP.mult, op1=OP.add)
            for t in range(TT):
                r = _rows(t)
                sig = small.tile([128, 512], BF16, tag="sig")
                nc.scalar.activation(sig[0:r, :], oh_s[0:r, t, :],
                                     ACTF.Sigmoid, scale=c1702[0:r, :])
                nc.vector.tensor_tensor(oh_s[0:r, t, :], oh_s[0:r, t, :],
                                        sig[0:r, :], OP.mult)
                for j in range(4):
                    pt = psT.tile([128, 128], BF16, tag="tp")
                    nc.tensor.transpose(pt[:, 0:r],
                                        oh_s[0:r, t, j * 128:(j + 1) * 128],
                                        id_bf[0:r, 0:r])
                    nc.vector.tensor_copy(
                        oht[:, sl * 4 + j, t * 128:t * 128 + r], pt[:, 0:r])
        if _DEBUG:
            nc.gpsimd.dma_start(dbg["dbg_oht"][:], oht[:])

        # ---- r2 routing ---------------------------------------------------
        routing(oht, FSUB, r2wT, r2b_row, r2_tm, r2T)
        if _DEBUG:
            nc.gpsimd.dma_start(dbg["dbg_r2"][:], r2_tm[:])

        # ---- tail stage + residual + store -------------------------------
        out_s = big.tile([128, TT, 384], F32, tag="ln")
        for dsl in range(2):
            d0 = dsl * 384
            for t in range(TT):
                r = _rows(t)
                pb = psB.tile([128, 512], F32, tag="mm2")
                nc.tensor.matmul(pb[0:r, 0:384],
                                 r2T[:, t * 128:t * 128 + r],
                                 bt_stack[:, d0:d0 + 384],
                                 start=True, stop=True)
                x1s = small.tile([128, 384], F32, tag="x1s")
                nc.scalar.dma_start(x1s[0:r, :], x1_dram[0:r, t, d0:d0 + 384])
                nc.vector.tensor_tensor(out_s[0:r, t, :], pb[0:r, 0:384],
                                        x1s[0:r, :], OP.add)
            for e in range(E1):
                wtc = dramS.tile([384, DF], BF16, tag="wtc")
                if e == 0:
                    nc.gpsimd.dma_start(
                        wtc[:], w["c_proj_w"][:][d0:d0 + 384, :])
                else:
                    nc.gpsimd.dma_start(
                        wtc[:], etw_flat[(e - 1) * D + d0:
                                         (e - 1) * D + d0 + 384, :])
                # distinct tags: 4 stream buffers in flight so expert e+1's
                # weight DMA overlaps expert e's matmuls (was a 5us PE gap
                # + HAM re-throttle per expert)
                wch0 = strm.tile([128, 12, 384], BF16, tag="s2a")
                nc.sync.dma_start_transpose(wch0[:], wtc[0:384, 0:12 * 128])
                wch1 = strm.tile([128, 12, 384], BF16, tag="s2b")
                nc.sync.dma_start_transpose(wch1[:], wtc[0:384, 12 * 128:])
                for t in range(TT):
                    r = _rows(t)
                    pa = psA.tile([128, 512], F32, tag="mm")
                    for s in range(FSUB):
                        wc = wch0 if s < 12 else wch1
                        nc.tensor.matmul(
                            pa[0:r, 0:384],
                            oht[:, s, t * 128:t * 128 + r],
                            wc[:, s % 12, :],
                            start=(s == 0), stop=(s == FSUB - 1))
                    nc.vector.scalar_tensor_tensor(
                        out=out_s[0:r, t, :], in0=pa[0:r, 0:384],
                        scalar=r2_tm[0:r, t, e:e + 1],
                        in1=out_s[0:r, t, :], op0=OP.mult, op1=OP.add)
            for t in range(TT):
                r = _rows(t)
                nc.scalar.dma_start(
                    out_flat[t * 128:t * 128 + r, d0:d0 + 384],
                    out_s[0:r, t, :])


# --------------------------------------------------------------------------
_cache = {}


def _get_nc():
    if 'nc' not in _cache:
        _cache['nc'] = build()
    return _cache['nc']


def _run(inputs, trace=False, trace_kwargs=None):
    nc = _get_nc()
    full = {k: np.ascontiguousarray(np.asarray(v), dtype=np.float32)
            for k, v in inputs.items()}
    in_maps = []
    for c in range(NCORES):
        m = {k: v for k, v in full.items() if k != 'x'}
        m['x'] = np.ascontiguousarray(full['x'][:, c * B:(c + 1) * B, :])
        in_maps.append(m)
    res = run_bass_kernel_spmd(nc, in_maps, core_ids=list(range(NCORES)),
                               trace=trace, **(trace_kwargs or {}))
    out = np.concatenate([res.results[c]['out'] for c in range(NCORES)],
                         axis=1)
    return out, res


def kernel(**inputs) -> np.ndarray:
    out, _ = _run(inputs, trace=False)
    return out



# revision 23
# speedup vs baseline: 1.4288x; 1.3559x over previous
"""AdapterFormer block (MHA + 5-branch soft-MoE FFN) on 8 TRN2 NeuronCores.

Data-parallel over batch (dim 1 of x): 64 -> 8 per core, weights
replicated, zero collectives.  Per-core tokens: 1576 = 197*8 (l-major,
tok = l*8 + b), 13 partition-tiles of 128 (last tile 40 valid rows).

Activations alternate between token-major (tok on partitions: LN stats,
softmax routing, per-token combines, residuals) and feature-major (feat
on partitions: matmul contraction layout).

The 90MB of expert/FFN weights are pre-quantized host-side to fp8e4
(x128 exact power-of-two scale, undone in the per-token routing
scalars) and shipped as uint16 pair-views.  On-device they stream
DRAM->SBUF through XBAR transpose-DMAs on BOTH HWDGE rings (sync +
scalar), and the head/tail matmuls run in DoubleRow fp8 mode (2 rows /
cycle).  The pair-transpose makes SBUF weights byte-interleaved along
the contraction axis (d = 256g + 2p + i); the matching activations are
produced by stride-2-column PE transposes into [128, G, 2, NT] tiles.

QKV/out_proj weights load as f32 rows on the gpsimd ring and are
PE-transposed on-chip (bf16); attention math is bf16.
"""
import os
import sys

import numpy as np

sys.path.insert(0, '/opt/trn_rl_repo')

import concourse.bass as bass
import concourse.mybir as mybir
import concourse.tile as tile
from concourse.bass_utils import run_bass_kernel_spmd
from concourse.masks import make_identity

F32 = mybir.dt.float32
BF16 = mybir.dt.bfloat16
F8E4 = mybir.dt.float8e4
U16 = mybir.dt.uint16
AX = mybir.AxisListType.X
OP = mybir.AluOpType
ACTF = mybir.ActivationFunctionType
DROW = mybir.MatmulPerfMode.DoubleRow

L, BT, D = 197, 64, 768
H, HD = 12, 64
E = 4
E1 = E + 1
DF = 4 * D                       # 3072
NCORES = 8
B = BT // NCORES                 # 8
NT = L * B                       # 1576
NTP = 1584                       # NT padded to 16 (DoubleRow lhsT stride
                                 # must be 16B-aligned: ISA s3_lw_dual_fp8)
TT = (NT + 127) // 128           # 13
DSUB = D // 128                  # 6
DG = D // 256                    # 3 double-row groups (head contraction)
FG = DF // 256                   # 12 double-row groups (tail contraction)
FSUB = DF // 128                 # 24
QKV3 = 3 * D
LROWS = NT - (TT - 1) * 128      # 40
EPS = 1e-5
SCALE = 0.125                    # 1/sqrt(HD)
WSC = 1.0 / 128.0                # undo host-side x128 fp8 weight scale
NCH = [(0, 512), (512, 512), (1024, 512), (1536, NT - 1536)]  # tok chunks

_DEBUG = bool(int(os.environ.get('BASS_KERNEL_DEBUG', '0')))


def _rows(t):
    return 128 if t < TT - 1 else LROWS


def _iv(ap2d, g, i):
    """Stride-2 column view: 128 cols d = 256g + 2p + i of a row-major AP."""
    return ap2d[:, g * 256:(g + 1) * 256].rearrange(
        "a (c i) -> a i c", i=2)[:, i, :]


def _f8pairs(u16ap):
    """[128, N] u16 AP -> [128, 2, N] fp8 moving-operand AP (byte pairs)."""
    n = u16ap.shape[-1]
    return u16ap.bitcast(F8E4).rearrange("p (f i) -> p i f", i=2)


# --------------------------------------------------------------------------
# walrus here rejects instructions with >1 sync-wait entry ("Too many sync
# wait commands"); move extra waits onto single-wait NoOps on the same queue.
_ws_ctr = [0]


def _fix_multiwait(nc, max_waits=1):
    for fn in nc.m.functions:
        for blk in fn.blocks:
            insts = list(blk.instructions)
            out = []
            changed = False
            for inst in insts:
                si = inst.sync_info
                waits = list(si.on_wait) if (si is not None and si.on_wait) else []
                if len(waits) > max_waits:
                    extra, keep = waits[:-max_waits], waits[-max_waits:]
                    for i in range(0, len(extra), max_waits):
                        _ws_ctr[0] += 1
                        out.append(mybir.InstNoOp(
                            name=f"I-ws{_ws_ctr[0]}",
                            sync_info=mybir.SyncInfo(
                                on_wait=list(extra[i:i + max_waits]),
                                on_update=[]),
                            bass_nofuse=True,
                            engine=inst.engine,
                        ))
                    si.on_wait = keep
                    inst.sync_info = si
                    changed = True
                out.append(inst)
            if changed:
                blk.instructions = out
    return nc


# --------------------------------------------------------------------------
def build():
    nc = bass.Bass("TRN2", target_bir_lowering=False, debug=False,
                   num_devices=NCORES)

    x_ext = nc.declare_dram_parameter("x", [L, B, D], F32, isOutput=False)
    w = {}
    for name, shape, dt in [
        ("ln1_g", [D], F32), ("ln1_b", [D], F32),
        ("in_proj_w", [QKV3, D], F32), ("in_proj_b", [QKV3], F32),
        ("out_proj_w", [D, D], F32), ("out_proj_b", [D], F32),
        ("ln2_g", [D], F32), ("ln2_b", [D], F32),
        ("c_fc_b", [DF], F32),
        ("c_proj_b", [D], F32),
        ("eh_b", [E, DF], F32),
        ("et_b", [E, D], F32),
        ("r1_w", [E1, D], F32), ("r1_b", [E1], F32),
        ("r2_w", [E1, DF], F32), ("r2_b", [E1], F32),
        # host-prepared fp8 weights (x128), stacked [base, e0..e3], viewed
        # as uint16 byte-pairs along the contraction axis
        ("wh8", [E1 * DF, D // 2], U16),
        ("wt8", [E1 * D, DF // 2], U16),
    ]:
        w[name] = nc.declare_dram_parameter(name, shape, dt, isOutput=False)
    out_ext = nc.declare_dram_parameter("out", [L, B, D], F32, isOutput=True)
    out_flat = out_ext[:].rearrange("l b d -> (l b) d")
    x_flat = x_ext[:].rearrange("l b d -> (l b) d")

    with tile.TileContext(nc) as tc:
        _body(nc, tc, x_flat, w, out_flat)

    _fix_multiwait(nc)
    return nc


def _body(nc, tc, x_flat, w, out_flat):
    from contextlib import ExitStack
    with ExitStack() as ctx:
        big = ctx.enter_context(tc.tile_pool(name="big", bufs=1))
        strm = ctx.enter_context(tc.tile_pool(name="strm", bufs=2))
        const = ctx.enter_context(tc.tile_pool(name="const", bufs=1))
        small = ctx.enter_context(tc.tile_pool(name="small", bufs=2))
        attnp = ctx.enter_context(tc.tile_pool(name="attnp", bufs=2))
        psA = ctx.enter_context(tc.tile_pool(name="psA", bufs=2,
                                             space="PSUM"))
        psB = ctx.enter_context(tc.tile_pool(name="psB", bufs=2,
                                             space="PSUM"))
        psT = ctx.enter_context(tc.tile_pool(name="psT", bufs=2,
                                             space="PSUM"))
        psS = ctx.enter_context(tc.tile_pool(name="psS", bufs=2,
                                             space="PSUM"))
        # phase-1 pools: QKV/out_proj weight staging; closed after the
        # attention block so the head/tail weight streams reuse the space
        p1 = ExitStack()
        wrows = p1.enter_context(tc.tile_pool(name="wrows", bufs=6))
        crows = p1.enter_context(tc.tile_pool(name="crows", bufs=2))
        wqp = p1.enter_context(tc.tile_pool(name="wqp", bufs=2))
        woutp = p1.enter_context(tc.tile_pool(name="woutp", bufs=1))

        # ---- constants ----------------------------------------------------
        id_bf = const.tile([128, 128], BF16)
        make_identity(nc, id_bf[:])
        id_f32 = const.tile([128, 128], F32)
        make_identity(nc, id_f32[:])
        ones_bf = const.tile([1, 128], BF16)
        nc.vector.memset(ones_bf[:], 1.0)
        eps_col = const.tile([128, 1], F32)
        nc.vector.memset(eps_col[:], EPS)
        c1702 = const.tile([128, 1], F32)
        nc.vector.memset(c1702[:], 1.702)

        # ---- tiny const rows first (scalar ring: lands in ~2us so their
        # PE transposes never gate on the bulk x traffic) ------------------
        def row_chunk(name, c0):
            row = crows.tile([1, D], F32, tag="crow")
            nc.scalar.dma_start(row[:], w[name][:].rearrange(
                "(a d) -> a d", a=1)[0:1, c0 * D:(c0 + 1) * D])
            return row

        def cols_from(row, dst, s0, n):
            for s in range(n):
                pt = psT.tile([128, 1], F32, tag="tp")
                nc.tensor.transpose(pt[:], row[0:1, s * 128:(s + 1) * 128],
                                    id_f32[0:1, 0:1])
                nc.vector.tensor_copy(dst[:, s0 + s:s0 + s + 1], pt[:])

        def cols_from_iv(row, dst):
            # interleaved cols: dst[p, g, i] = row[256g + 2p + i]
            for g in range(DG):
                for i in range(2):
                    pt = psT.tile([128, 1], F32, tag="tp")
                    nc.tensor.transpose(pt[:], _iv(row, g, i),
                                        id_f32[0:1, 0:1])
                    nc.vector.tensor_copy(dst[:, g, i:i + 1], pt[:])

        row_ln1g = row_chunk("ln1_g", 0)
        row_ln1b = row_chunk("ln1_b", 0)
        rows_ipb = [row_chunk("in_proj_b", c) for c in range(3)]

        # ---- load x token-major, split across BOTH HWDGE rings -----------
        x_tm = big.tile([128, TT, D], F32, tag="x1")
        nc.vector.memset(x_tm[:, TT - 1, :], 0.0)
        for t in range(TT):
            r = _rows(t)
            eng = nc.scalar if t % 2 == 0 else nc.sync
            eng.dma_start(x_tm[0:r, t, :], x_flat[t * 128: t * 128 + r, :])

        # ln2 g/b rows: behind the x halves on the scalar ring (~58us) --
        row_ln2g = row_chunk("ln2_g", 0)
        row_ln2b = row_chunk("ln2_b", 0)

        # ---- QKV + out_proj weight rows as f32 on the gpsimd ring (idle
        # at start), streamed through a 6-slot pool in consumption order --
        M_ORDER = [mt + 6 * j for mt in range(DSUB) for j in range(3)]
        wrow_t = {}
        for m in M_ORDER:
            wt = wrows.tile([128, D], F32, tag="wrow")
            nc.gpsimd.dma_start(wt[:],
                                w["in_proj_w"][:][m * 128:(m + 1) * 128, :])
            wrow_t[m] = wt
        wout_t = {}
        for m in range(DSUB):
            wt = wrows.tile([128, D], F32, tag="wrow")
            nc.gpsimd.dma_start(wt[:],
                                w["out_proj_w"][:][m * 128:(m + 1) * 128, :])
            wout_t[m] = wt

        ln1g = const.tile([128, DSUB], F32, tag="col_ln1g")
        cols_from(row_ln1g, ln1g, 0, DSUB)
        ln1b = const.tile([128, DSUB], F32, tag="col_ln1b")
        cols_from(row_ln1b, ln1b, 0, DSUB)
        bqkv = const.tile([128, 18], F32, tag="col_bqkv")
        for c in range(3):
            cols_from(rows_ipb[c], bqkv, 6 * c, DSUB)

        # ---- layernorm helper --------------------------------------------
        lnt = big.tile([128, D], BF16, tag="lnt")

        def layernorm_stats(t):
            r = _rows(t)
            xs = x_tm[0:r, t, :]
            s1 = small.tile([128, 1], F32, tag="ln_s1")
            nc.vector.reduce_sum(s1[0:r, :], xs, AX)
            sq = small.tile([128, 1], F32, tag="ln_sq")
            # Square's elementwise output is throwaway (only accum_out
            # matters); dump it into lnt, overwritten below anyway.
            nc.scalar.activation(lnt[0:r, :], xs, ACTF.Square,
                                 accum_out=sq[0:r, :])
            mu = small.tile([128, 1], F32, tag="ln_mu")
            nc.vector.tensor_scalar(mu[0:r, :], s1[0:r, :], 1.0 / D,
                                    None, OP.mult)
            mu2 = small.tile([128, 1], F32, tag="ln_mu2")
            nc.vector.tensor_tensor(mu2[0:r, :], mu[0:r, :], mu[0:r, :],
                                    OP.mult)
            var = small.tile([128, 1], F32, tag="ln_var")
            nc.vector.scalar_tensor_tensor(
                out=var[0:r, :], in0=sq[0:r, :], scalar=1.0 / D,
                in1=mu2[0:r, :], op0=OP.mult, op1=OP.subtract)
            sd = small.tile([128, 1], F32, tag="ln_sd")
            nc.scalar.activation(sd[0:r, :], var[0:r, :], ACTF.Sqrt,
                                 bias=eps_col[0:r, :])
            a_col = small.tile([128, 1], F32, tag="ln_a")
            nc.vector.reciprocal(a_col[0:r, :], sd[0:r, :])
            b_col = small.tile([128, 1], F32, tag="ln_b")
            nc.vector.scalar_tensor_tensor(
                out=b_col[0:r, :], in0=mu[0:r, :], scalar=-1.0,
                in1=a_col[0:r, :], op0=OP.mult, op1=OP.mult)
            nc.scalar.activation(lnt[0:r, :], xs, ACTF.Identity,
                                 bias=b_col[0:r, :], scale=a_col[0:r, :])
            return r

        # ---- LN1: natural layout, bf16 (feeds QKV) -----------------------
        ln_fm = big.tile([128, DSUB, NT], BF16, tag="ln")
        for t in range(TT):
            r = layernorm_stats(t)
            for s in range(DSUB):
                pt = psT.tile([128, 128], BF16, tag="tp")
                nc.tensor.transpose(pt[:, 0:r],
                                    lnt[0:r, s * 128:(s + 1) * 128],
                                    id_bf[0:r, 0:r])
                nc.vector.tensor_scalar(
                    ln_fm[:, s, t * 128:t * 128 + r], pt[:, 0:r],
                    ln1g[:, s:s + 1], ln1b[:, s:s + 1],
                    OP.mult, OP.add)

        # ln2 cols now (rows arrived behind x): interleaved layout --------
        ln2g = const.tile([128, DG, 2], F32, tag="col_ln2g")
        cols_from_iv(row_ln2g, ln2g)
        ln2b = const.tile([128, DG, 2], F32, tag="col_ln2b")
        cols_from_iv(row_ln2b, ln2b)

        opb_row = const.tile([1, D], BF16)
        nc.gpsimd.dma_start(opb_row[:], w["out_proj_b"][:].rearrange("(a d) -> a d", a=1))
        r1b_row = const.tile([1, E1], BF16)
        nc.gpsimd.dma_start(r1b_row[:], w["r1_b"][:].rearrange("(a e) -> a e", a=1))
        r2b_row = const.tile([1, E1], BF16)
        nc.gpsimd.dma_start(r2b_row[:], w["r2_b"][:].rearrange("(a e) -> a e", a=1))
        bt_stack = const.tile([E1, D], BF16)
        nc.gpsimd.dma_start(bt_stack[0:1, :],
                            w["c_proj_b"][:].rearrange("(a d) -> a d", a=1))
        nc.gpsimd.dma_start(bt_stack[1:, :], w["et_b"][:])

        r1_tm = const.tile([128, TT, E1], F32)
        r2_tm = const.tile([128, TT, E1], F32)
        r1T = const.tile([E1, NT], BF16)
        r2T = const.tile([E1, NT], BF16)

        # ---- QKV + attention, interleaved per head-pair tile mt ----------
        o_fm = big.tile([128, DSUB, NT], BF16, tag="oa")
        o_lb = o_fm[:].rearrange("p m (l b) -> p m l b", b=B)
        LT = [(0, 128), (128, L - 128)]
        for mt in range(DSUB):
            qkv5 = big.tile([128, 3, NT], BF16, tag="qk")
            for j, m in enumerate([mt, 6 + mt, 12 + mt]):
                # PE-transpose the f32 weight rows into matmul layout
                wq = wqp.tile([128, DSUB, 128], BF16, tag=f"wq{j}")
                for s in range(DSUB):
                    pt = psT.tile([128, 128], F32, tag="tp")
                    nc.tensor.transpose(
                        pt[:], wrow_t[m][:, s * 128:(s + 1) * 128],
                        id_f32[:])
                    nc.vector.tensor_copy(wq[:, s, :], pt[:])
                for c0, cn in NCH:
                    pa = psA.tile([128, 512], F32, tag="mm")
                    for s in range(DSUB):
                        nc.tensor.matmul(pa[:, 0:cn], wq[:, s, :],
                                         ln_fm[:, s, c0:c0 + cn],
                                         start=(s == 0), stop=(s == DSUB - 1))
                    if j == 0:
                        nc.vector.tensor_scalar(
                            qkv5[:, j, c0:c0 + cn], pa[:, 0:cn],
                            bqkv[:, m:m + 1], SCALE, OP.add, OP.mult)
                    else:
                        nc.vector.tensor_scalar(
                            qkv5[:, j, c0:c0 + cn], pa[:, 0:cn],
                            bqkv[:, m:m + 1], None, OP.add)
            qkv_lb = qkv5[:].rearrange("p j (l b) -> p j l b", b=B)
            v_all = small.tile([128, B, 2, 128], BF16, tag="v_tm")
            for b in range(B):
                vT2 = qkv_lb[:, 2, :, b]              # [128, 197] both heads
                for jj, (m0, mc) in enumerate(LT):
                    pt = psT.tile([128, 128], BF16, tag="tp")
                    nc.tensor.transpose(pt[0:mc, :], vT2[:, m0:m0 + mc],
                                        id_bf[:])
                    nc.vector.tensor_copy(v_all[0:mc, b, jj, :],
                                          pt[0:mc, :])
            for h in (2 * mt, 2 * mt + 1):
                po = (h % 2) * 64
                for b in range(B):
                    v_tm = v_all[:, b, :, :]
                    qT = qkv_lb[po:po + 64, 0, :, b]
                    kT = qkv_lb[po:po + 64, 1, :, b]
                    attn = attnp.tile([128, 2, L], BF16, tag="attn")
                    rs = attnp.tile([128, 2], F32, tag="attn_rs")
                    for i, (l0, lc) in enumerate(LT):
                        ps = psS.tile([128, L], F32, tag="att")
                        nc.tensor.matmul(ps[0:lc, :], qT[:, l0:l0 + lc], kT,
                                         start=True, stop=True)
                        sums = attnp.tile([128, 1], F32, tag="attn_sum")
                        nc.scalar.activation(attn[0:lc, i, :], ps[0:lc, :],
                                             ACTF.Exp,
                                             accum_out=sums[0:lc, :])
                        nc.vector.reciprocal(rs[0:lc, i:i + 1],
                                             sums[0:lc, :])
                        nc.vector.tensor_scalar(
                            attn[0:lc, i, :], attn[0:lc, i, :],
                            rs[0:lc, i:i + 1], None, OP.mult)
                    attnT = attnp.tile([128, 2, L], BF16, tag="attnT")
                    for jj, (m0, mc) in enumerate(LT):
                        for i, (l0, lc) in enumerate(LT):
                            pt = psT.tile([128, 128], BF16, tag="tp")
                            nc.tensor.transpose(
                                pt[0:mc, 0:lc], attn[0:lc, i, m0:m0 + mc],
                                id_bf[0:lc, 0:lc])
                            nc.vector.tensor_copy(
                                attnT[0:mc, jj, l0:l0 + lc], pt[0:mc, 0:lc])
                    po_ps = psB.tile([64, L], F32, tag="mm2")
                    for jj, (m0, mc) in enumerate(LT):
                        nc.tensor.matmul(po_ps[:],
                                         v_tm[0:mc, jj, po:po + 64],
                                         attnT[0:mc, jj, :],
                                         start=(jj == 0), stop=(jj == 1))
                    nc.vector.tensor_copy(o_lb[po:po + 64, mt, :, b],
                                          po_ps[:])

        # ---- out_proj (token-major) + residual into x_tm ------------------
        woutT = woutp.tile([128, DSUB, D], BF16, tag="woutT")
        for mrow in range(DSUB):
            for s in range(DSUB):
                pt = psT.tile([128, 128], F32, tag="tp")
                nc.tensor.transpose(
                    pt[:], wout_t[mrow][:, s * 128:(s + 1) * 128],
                    id_f32[:])
                nc.vector.tensor_copy(
                    woutT[:, s, mrow * 128:(mrow + 1) * 128], pt[:])
        DCH = [(0, 512), (512, 256)]
        for t in range(TT):
            r = _rows(t)
            for c0, cn in DCH:
                pa = psA.tile([128, 512], F32, tag="mm")
                for s in range(DSUB):
                    nc.tensor.matmul(
                        pa[0:r, 0:cn], o_fm[:, s, t * 128:t * 128 + r],
                        woutT[:, s, c0:c0 + cn],
                        start=(s == 0), stop=False)
                nc.tensor.matmul(pa[0:r, 0:cn], ones_bf[0:1, 0:r],
                                 opb_row[0:1, c0:c0 + cn],
                                 start=False, stop=True)
                nc.vector.tensor_tensor(
                    x_tm[0:r, t, c0:c0 + cn], pa[0:r, 0:cn],
                    x_tm[0:r, t, c0:c0 + cn], OP.add)

        # close phase-1 pools: head/tail weight streams reuse the space
        p1.close()
        stws = ctx.enter_context(tc.tile_pool(name="stws", bufs=2))

        # ---- LN2: interleaved layout, fp8, into the dead qkv5 slot -------
        ln2_fm = big.tile([128, DG, 2, NTP], F8E4, tag="qk")
        for t in range(TT):
            r = layernorm_stats(t)
            for g in range(DG):
                for i in range(2):
                    pt = psT.tile([128, 128], BF16, tag="tp")
                    nc.tensor.transpose(pt[:, 0:r],
                                        _iv(lnt[0:r, :], g, i),
                                        id_bf[0:r, 0:r])
                    nc.vector.tensor_scalar(
                        ln2_fm[:, g, i, t * 128:t * 128 + r], pt[:, 0:r],
                        ln2g[:, g, i:i + 1], ln2b[:, g, i:i + 1],
                        OP.mult, OP.add)

        # routing weights transposed, interleaved to match ln2_fm / oht
        r1wT = const.tile([128, DG, 2, E1], BF16)
        rw1 = stws.tile([E1, D], BF16, tag="rw")
        nc.gpsimd.dma_start(rw1[:], w["r1_w"][:])
        for g in range(DG):
            for i in range(2):
                pt = psT.tile([128, E1], BF16, tag="tp")
                nc.tensor.transpose(pt[:], _iv(rw1[:], g, i),
                                    id_bf[0:E1, 0:E1])
                nc.vector.tensor_copy(r1wT[:, g, i, :], pt[:])
        r2wT = const.tile([128, FG, 2, E1], BF16)
        for c in range(4):
            rw2 = stws.tile([E1, D], BF16, tag="rw")
            nc.gpsimd.dma_start(rw2[:], w["r2_w"][:][:, c * D:(c + 1) * D])
            for gg in range(DG):
                for i in range(2):
                    pt = psT.tile([128, E1], BF16, tag="tp")
                    nc.tensor.transpose(pt[:], _iv(rw2[:], gg, i),
                                        id_bf[0:E1, 0:E1])
                    nc.vector.tensor_copy(r2wT[:, c * DG + gg, i, :], pt[:])

        # ---- routing helper (token-major logits, no max-sub: tiny logits)
        # r_tm is written pre-scaled by WSC (undoes the x128 fp8 weight
        # scale at the combine step); rT (for the bias matmuls) unscaled.
        def routing(act_fm, ng, wT, b_row, r_tm, rT):
            for t in range(TT):
                r = _rows(t)
                pr = psB.tile([128, 512], F32, tag="mm2")
                k = 0
                for g in range(ng):
                    for i in range(2):
                        nc.tensor.matmul(pr[0:r, 0:E1],
                                         act_fm[:, g, i,
                                                t * 128:t * 128 + r],
                                         wT[:, g, i, :],
                                         start=(k == 0), stop=False)
                        k += 1
                nc.tensor.matmul(pr[0:r, 0:E1], ones_bf[0:1, 0:r],
                                 b_row[0:1, :], start=False, stop=True)
                e_t = small.tile([128, E1], F32, tag="rt_exp")
                sums = small.tile([128, 1], F32, tag="rt_sum")
                nc.scalar.activation(e_t[0:r, :], pr[0:r, 0:E1], ACTF.Exp,
                                     accum_out=sums[0:r, :])
                rsum = small.tile([128, 1], F32, tag="rt_rsum")
                nc.vector.reciprocal(rsum[0:r, :], sums[0:r, :])
                rr = small.tile([128, E1], F32, tag="rt_rr")
                nc.vector.tensor_scalar(rr[0:r, :], e_t[0:r, :],
                                        rsum[0:r, :], None, OP.mult)
                nc.vector.tensor_scalar(r_tm[0:r, t, :], rr[0:r, :], WSC,
                                        None, OP.mult)
                ptb = psT.tile([E1, 128], F32, tag="tp")
                nc.tensor.transpose(ptb[:, 0:r], rr[0:r, :],
                                    id_f32[0:r, 0:r])
                nc.vector.tensor_copy(rT[:, t * 128:t * 128 + r],
                                      ptb[:, 0:r])

        routing(ln2_fm, DG, r1wT, r1b_row, r1_tm, r1T)

        # ---- head stage ---------------------------------------------------
        # oh_s (token-major, bf16 accum) -> quickgelu -> transpose into oht
        # (fp8, interleaved f layout for the tail DoubleRow contraction)
        oht = big.tile([128, FG, 2, NTP], F8E4, tag="oht")
        oh_s = big.tile([128, TT, 512], BF16, tag="oa")
        wh8r = w["wh8"][:]
        for sl in range(DF // 512):
            bh = stws.tile([E1, 512], BF16, tag="bh")
            nc.gpsimd.dma_start(bh[0:1, :], w["c_fc_b"][:].rearrange(
                "(a f) -> a f", a=1)[0:1, sl * 512:(sl + 1) * 512])
            nc.gpsimd.dma_start(bh[1:, :],
                                w["eh_b"][:][:, sl * 512:(sl + 1) * 512])
            for t in range(TT):
                r = _rows(t)
                pb = psB.tile([128, 512], F32, tag="mm2")
                nc.tensor.matmul(pb[0:r, :], r1T[:, t * 128:t * 128 + r],
                                 bh[:, :], start=True, stop=True)
                nc.vector.tensor_copy(oh_s[0:r, t, :], pb[0:r, :])
            for e in range(E1):
                # fp8 weight chunk, pair-transposed straight from DRAM;
                # alternate HWDGE rings (each is packet-rate-bound)
                wht = stws.tile([128, DG, 512], U16, tag="wh")
                ring = nc.sync if (sl * E1 + e) % 2 == 0 else nc.scalar
                ring.dma_start_transpose(
                    wht[:], wh8r[e * DF + sl * 512:e * DF + (sl + 1) * 512,
                                 :])
                for t in range(TT):
                    r = _rows(t)
                    pa = psA.tile([128, 512], F32, tag="mm")
                    for g in range(DG):
                        nc.tensor.matmul(
                            pa[0:r, :],
                            ln2_fm[:, g, :, t * 128:t * 128 + r],
                            _f8pairs(wht[:, g, :]),
                            start=(g == 0), stop=(g == DG - 1),
                            perf_mode=DROW)
                    nc.vector.scalar_tensor_tensor(
                        out=oh_s[0:r, t, :], in0=pa[0:r, :],
                        scalar=r1_tm[0:r, t, e:e + 1],
                        in1=oh_s[0:r, t, :], op0=OP.mult, op1=OP.add)
            for t in range(TT):
                r = _rows(t)
                sig = small.tile([128, 512], BF16, tag="sig")
                nc.scalar.activation(sig[0:r, :], oh_s[0:r, t, :],
                                     ACTF.Sigmoid, scale=c1702[0:r, :])
                nc.vector.tensor_tensor(oh_s[0:r, t, :], oh_s[0:r, t, :],
                                        sig[0:r, :], OP.mult)
                for gg in range(2):
                    for i in range(2):
                        pt = psT.tile([128, 128], BF16, tag="tp")
                        nc.tensor.transpose(
                            pt[:, 0:r],
                            _iv(oh_s[0:r, t, :], gg, i),
                            id_bf[0:r, 0:r])
                        nc.vector.tensor_copy(
                            oht[:, sl * 2 + gg, i, t * 128:t * 128 + r],
                            pt[:, 0:r])

        # ---- r2 routing ---------------------------------------------------
        routing(oht, FG, r2wT, r2b_row, r2_tm, r2T)

        # ---- tail stage + residual + store -------------------------------
        out_s = big.tile([128, TT, 384], F32, tag="ln")
        wt8r = w["wt8"][:]
        for dsl in range(2):
            d0 = dsl * 384
            for t in range(TT):
                r = _rows(t)
                pb = psB.tile([128, 512], F32, tag="mm2")
                nc.tensor.matmul(pb[0:r, 0:384],
                                 r2T[:, t * 128:t * 128 + r],
                                 bt_stack[:, d0:d0 + 384],
                                 start=True, stop=True)
                nc.vector.tensor_tensor(out_s[0:r, t, :], pb[0:r, 0:384],
                                        x_tm[0:r, t, d0:d0 + 384], OP.add)
            for e in range(E1):
                wtt = stws.tile([128, FG, 384], U16, tag="wt")
                ring = nc.sync if (dsl * E1 + e) % 2 == 0 else nc.scalar
                ring.dma_start_transpose(
                    wtt[:], wt8r[e * D + d0:e * D + d0 + 384, :])
                for t in range(TT):
                    r = _rows(t)
                    pa = psA.tile([128, 512], F32, tag="mm")
                    for g in range(FG):
                        nc.tensor.matmul(
                            pa[0:r, 0:384],
                            oht[:, g, :, t * 128:t * 128 + r],
                            _f8pairs(wtt[:, g, :]),
                            start=(g == 0), stop=(g == FG - 1),
                            perf_mode=DROW)
                    nc.vector.scalar_tensor_tensor(
                        out=out_s[0:r, t, :], in0=pa[0:r, 0:384],
                        scalar=r2_tm[0:r, t, e:e + 1],
                        in1=out_s[0:r, t, :], op0=OP.mult, op1=OP.add)
            for t in range(TT):
                r = _rows(t)
                eng = nc.scalar if t % 2 == 0 else nc.sync
                eng.dma_start(
                    out_flat[t * 128:t * 128 + r, d0:d0 + 384],
                    out_s[0:r, t, :])


# --------------------------------------------------------------------------
_cache = {}


def _get_nc():
    if 'nc' not in _cache:
        _cache['nc'] = build()
    return _cache['nc']


def _prep_fp8(full):
    """Host-side fp8e4 pre-quantization of the expert/FFN weights.

    Exact x128 (2^7) scale lifts the ~N(0, 0.02) weights out of the
    e4m3 subnormal range; the kernel undoes it in the routing scalars.
    Stacked [base, e0..e3] and bit-viewed as uint16 pairs so the XBAR
    transpose-DMA (16-bit-only) can move them.
    """
    import ml_dtypes
    wh = np.concatenate([full['c_fc_w'][None], full['eh_w']], axis=0)
    wt = np.concatenate([full['c_proj_w'][None], full['et_w']], axis=0)
    wh8 = (wh * 128.0).astype(ml_dtypes.float8_e4m3).reshape(E1 * DF, D)
    wt8 = (wt * 128.0).astype(ml_dtypes.float8_e4m3).reshape(E1 * D, DF)
    return (np.ascontiguousarray(wh8).view(np.uint16),
            np.ascontiguousarray(wt8).view(np.uint16))


def _run(inputs, trace=False, trace_kwargs=None):
    nc = _get_nc()
    full = {k: np.ascontiguousarray(np.asarray(v), dtype=np.float32)
            for k, v in inputs.items()}
    wh8, wt8 = _prep_fp8(full)
    base = {k: v for k, v in full.items()
            if k not in ('x', 'c_fc_w', 'eh_w', 'c_proj_w', 'et_w')}
    base['wh8'] = wh8
    base['wt8'] = wt8
    in_maps = []
    for c in range(NCORES):
        m = dict(base)
        m['x'] = np.ascontiguousarray(full['x'][:, c * B:(c + 1) * B, :])
        in_maps.append(m)
    res = run_bass_kernel_spmd(nc, in_maps, core_ids=list(range(NCORES)),
                               trace=trace, **(trace_kwargs or {}))
    out = np.concatenate([res.results[c]['out'] for c in range(NCORES)],
                         axis=1)
    return out, res


def kernel(**inputs) -> np.ndarray:
    out, _ = _run(inputs, trace=False)
    return out
